# revision 1
# baseline (speedup 1.0000x reference)
"""DGCNN forward kernel for Trainium2, data-parallel over 8 NeuronCores.

Structure of the problem (shapes hardcoded from the task spec):
  x          [1_000_000, 64]  fp32   node features, 10_000 graphs x 100 nodes
  batch      arange(N)//100          (graphs are contiguous 100-node blocks)
  4-layer MLP 64->64->64->64->34 with ReLU
  mean-pool over the FIRST 30 nodes of each graph  -> [10_000, 34]
  conv1d(1->16,k=5) + ReLU -> maxpool(2) -> conv1d(16->32,k=5) + ReLU
  flatten -> linear(352->2)

v3 design (cost-model driven, validated against the walrus compiler):
  * everything bf16 (x, weights, activations): halves DMA bytes; matmuls
    are bf16 (1 cycle/row at any N) accumulating into fp32 PSUM.
  * only the first 30 of every 100 nodes ship to the device; two node
    halves are packed into the 128 SBUF partitions with block-diagonal
    MLP weights (L4 puts half B at partitions 64:98 so the conv head
    can read it at a legal base partition).
  * tiles are [128, 1920] = 4 chunks x 480 cols, no inter-chunk pad.
  * relu(+bias) runs as 960-col pair ops. Only ACT and DVE can read
    PSUM (the Pool engine cannot - the cost model accepts it but the
    real compiler rejects it), so relus split between those two by a
    greedy wave-ordered planner; the Pool engine handles SBUF-side
    pooling halves (30->15->8) for the early tiles, with a low-latency
    all-DVE path for the last tiles so the drain phase stays short.
  * emission is software-pipelined in waves (L1 of tile w, L2 of w-D,
    ...) because engines dispatch strictly in order - buffer pools
    alone cannot create cross-tile overlap.
  * the conv head runs directly on the packed pooled layout (no unpack
    DMAs), interleaved into the last waves; conv1+maxpool uses the
    even/odd split with ACT relu + DVE scalar_tensor_tensor (max).
  * the final linear bias is added on host; head constants are DMA'd
    after the whole x stream (the serial SP queue is position=priority).
"""

import json

import numpy as np

# ---------------------------------------------------------------- constants
N = 1_000_000
G = 10_000
NODES_PER_G = 100
K = 30
F = 64
NCORE = 8
G_CORE = G // NCORE          # 1250 graphs per core
G_HALF = G_CORE // 2         # 625 graphs per packed half
G_HALF_PAD = 640             # padded to 10 tiles of 64 graphs
TILE_G = 64                  # graphs per half per tile
N_TILES = G_HALF_PAD // TILE_G
N_CHUNKS = 4                 # 480-col matmul chunks per tile
DATA_COLS = 480              # 16 graphs * 30 nodes
TILE_COLS = N_CHUNKS * DATA_COLS    # 1920
PS_STRIDE = 512              # psum chunk stride (bank = 512 fp32)
HEAD_COLS = 2 * G_HALF_PAD   # 1280 = A half cols then B half cols
LAST_G = G_HALF - (N_TILES - 1) * TILE_G   # 49 real graphs in tile 9

# const tensor column offsets (bf16, [128, NCOL])
OFF_W1, OFF_W2, OFF_W3 = 0, 128, 256
OFF_W4 = 384                      # 98 cols (A out 0:34, B out 64:98)
OFF_CE1, OFF_CO1 = 482, 610       # conv1 lhsT, t even/odd 0..15
OFF_CEM, OFF_COM = 738, 866       # conv1 lhsT, t 8..23
OFF_CE2, OFF_CO2 = 994, 1106     # conv1 lhsT, t 16..29 (112 cols)
OFF_C20, OFF_C21 = 1218, 1346    # conv2 lhsT t=0..3 / 4..7 (128 cols)
OFF_C22 = 1474                    # conv2 lhsT t=8..10 ([112, 96])
OFF_WO0, OFF_WO1, OFF_WO2 = 1570, 1572, 1574
NCOL = 1576
# fp32 bias tensor columns ([128, 8])
FB_B1, FB_B2, FB_B3, FB_B4, FB_C1B, FB_C2B = 0, 1, 2, 3, 4, 5


# ------------------------------------------------- walrus sync-wait workaround
def _split_sync_waits(bir: dict) -> dict:
    """The walrus build in this container accepts at most ONE sync-wait per
    instruction.  Hoist extra waits onto same-engine EventSemaphore carriers
    (the exact shape wait_ge() emits) inserted right before the instruction;
    engines dispatch in order so semantics are unchanged."""
    for fn in bir.get("functions", []):
        for bb in fn.get("blocks", []):
            out = []
            for inst in bb.get("instructions", []):
                si = inst.get("sync_info") or {}
                ow = si.get("on_wait") or []
                if len(ow) > 1:
                    for k, w in enumerate(ow[:-1]):
                        out.append(
                            {
                                "debug": inst.get("debug"),
                                "engine": inst["engine"],
                                "ins": [],
                                "name": f"{inst['name']}_hw{k}",
                                "opcode": "EventSemaphore",
                                "outs": [],
                                "sync_info": {"on_update": [], "on_wait": [w]},
                            }
                        )
                    si = dict(si)
                    si["on_wait"] = [ow[-1]]
                    inst = dict(inst)
                    inst["sync_info"] = si
                out.append(inst)
            bb["instructions"] = out
    return bir


_patch_installed = False


def _install_bir_patch():
    global _patch_installed
    if _patch_installed:
        return
    import concourse.bass as bass

    orig = bass.Bass.to_json_bytes
    if getattr(bass.Bass, "_ant_sync_wait_patch", False):
        _patch_installed = True
        return

    def patched(self) -> bytes:
        return json.dumps(_split_sync_waits(json.loads(orig(self)))).encode()

    bass.Bass.to_json_bytes = patched
    bass.Bass._ant_sync_wait_patch = True
    _patch_installed = True


# tuning knobs (read by _build_nc; override before first _get_nc call)
TUNE = {
    # per-tile relu engine map: two chars per layer (pair groups), 'a'/'d'/'p'.
    "emap": ["apapadap", "apdpapad", "apapadap", "apdpapad", "apapadap",
             "apdpapad", "apapadap", "apdpapad", "apapadap", "apdpapad"],
    "halve_eng": "d",         # pooling halve engine: 'd' or 'p'
    "head_at": {6: [(0, 0, 256)], 7: [(1, 0, 256)], 8: [(0, 256, 256)],
                9: [(1, 256, 256), (0, 512, 128), (1, 512, 128)]},
    # head engine maps: relu-after-max (3 bands), conv2-relu (3), fc copy (1)
    "h_relu1": "ddd",
    "h_relu2": "aaa",
    "h_cpy": "a",
    "ps_bufs": 4,
    "x0_split": 1,
    "pool_modes": "pppppppddd",
    "wave_depth": 2,
    "hp_bufs": 12,
    "x_queues": None,         # per-tile DMA queue ('s'/'a'/'d'), default all SP
}


def _plan_engines():
    """Greedy wave-ordered engine assignment.

    The real walrus compiler only lets ACT and DVE read PSUM, so relus and
    everything PSUM-sourced split between those two; the Pool engine takes
    SBUF-side pooling halves.
    """
    DEPTH = TUNE.get("wave_depth", 2)
    head_at = TUNE["head_at"]
    singles = TUNE.get("singles", False)
    dbias = TUNE.get("dve_bias", 0.0)
    cost_pair = ({"a": 585.0, "d": 625.0 + dbias} if singles else
                 {"a": 985.0, "d": 1125.0 + dbias})
    gstep = 1 if singles else 2
    load = {"a": 0.0, "d": 0.0, "p": 0.0}
    relu_eng = {}
    h2_eng = {}
    cpy_eng = {}

    def pick(costs, exclude=()):
        best, bc = None, None
        for e, c in costs.items():
            if e in exclude:
                continue
            v = load[e] + c
            if bc is None or v < bc:
                best, bc = e, v
        load[best] += costs[best]
        return best

    def plan_head(ci, C):
        # conv1 bands: se on ACT + stt on DVE (fixed split)
        load["a"] += 3 * (C * 0.8333 + 185)
        load["d"] += 3 * (C * 1.0417 + 125)
        for c2 in range(3):
            h2_eng[(ci, c2)] = pick({
                "d": C * 1.0417 + 125, "a": C * 0.8333 + 185,
            })
        cpy_eng[ci] = pick({"d": C * 1.0417 + 125, "a": C * 0.8333 + 185})

    hc_i = {}
    for w in range(N_TILES + 3 * DEPTH + 1):
        for li in range(4):
            i = w - li * DEPTH
            if 0 <= i < N_TILES:
                ncols = TILE_COLS if i < N_TILES - 1 else LAST_G * K
                first = None
                for gi, c0 in enumerate(range(0, 4, gstep)):
                    if c0 * DATA_COLS >= ncols:
                        continue
                    cols = min(gstep * DATA_COLS, ncols - c0 * DATA_COLS)
                    scale = cols / float(gstep * DATA_COLS)
                    costs = {e: c * scale for e, c in cost_pair.items()}
                    e = pick(costs, exclude=(first,) if first and not singles else ())
                    relu_eng[(i, li, gi)] = e
                    if first is None:
                        first = e
        ip = w - 3 * DEPTH - 1
        if 0 <= ip < N_TILES:
            if TUNE["pool_modes"][ip] == "p":
                # 2 cascaded halves + odd-col copy on Pool, reduce on DVE
                load["p"] += 2000.0 + 984.0 + 184.0
                load["d"] += 593.0
            else:
                load["d"] += 560.0 + 1060.0
            for (hf, c0, C) in head_at.get(ip, []):
                hc_i[(hf, c0)] = True
                plan_head((hf, c0), C)
    for hf, c0, C in TUNE.get("head_all", [
        (0, 0, 256), (1, 0, 256), (0, 256, 256),
        (1, 256, 256), (0, 512, 128), (1, 512, 128),
    ]):
        if (hf, c0) not in hc_i:
            plan_head((hf, c0), C)
    return relu_eng, h2_eng, cpy_eng


# ------------------------------------------------------------- device program
_NC_CACHE = {}


def _build_nc():
    """Build the per-core Bass program (identical on all 8 cores)."""
    _install_bir_patch()
    import concourse.bass as bass
    import concourse.tile as tile
    from concourse import mybir

    f32 = mybir.dt.float32
    bf16 = mybir.dt.bfloat16
    Relu = mybir.ActivationFunctionType.Relu
    ADD = mybir.AluOpType.add
    MAX = mybir.AluOpType.max
    AX = mybir.AxisListType.X

    nc = bass.Bass()
    xt = nc.dram_tensor("xt", [N_TILES, 128, TILE_COLS], bf16, kind="ExternalInput")
    cst = nc.dram_tensor("cst", [128, NCOL], bf16, kind="ExternalInput")
    cstf = nc.dram_tensor("cstf", [128, 8], f32, kind="ExternalInput")
    y = nc.dram_tensor("y", [2, HEAD_COLS], f32, kind="ExternalOutput")

    relu_eng_p, h2_eng_p, cpy_eng_p = _plan_engines()

    with tile.TileContext(nc) as tc:
        with (
            tc.tile_pool(name="persist", bufs=1) as persist,
            tc.tile_pool(name="xp", bufs=3) as xp,
            tc.tile_pool(name="hp", bufs=TUNE["hp_bufs"]) as hp,
            tc.tile_pool(name="hsb", bufs=2) as hsb,
        ):
            cstt = persist.tile([128, NCOL], bf16)
            cstf_t = persist.tile([128, 8], f32)
            # DMA order on the SP queue: MLP weights -> x tile 0 -> biases ->
            # head constants -> x tiles 1..9 (the serialized HWDGE FIFO makes
            # position = priority).
            nc.sync.dma_start(out=cstt[:, 0:OFF_CE1], in_=cst[:, 0:OFF_CE1])
            xq = TUNE.get("x_queues") or "s" * N_TILES
            q_eng = {"s": nc.sync, "a": nc.scalar, "d": nc.vector, "g": nc.gpsimd}
            xt_sb = []
            for i in range(N_TILES):
                xt_sb.append(xp.tile([128, TILE_COLS], bf16, name="xt_i"))
            nsp0 = TUNE.get("x0_split", 4)
            step0 = TILE_COLS // nsp0
            for cq in range(nsp0):
                q_eng[xq[0]].dma_start(
                    out=xt_sb[0][:, cq * step0 : (cq + 1) * step0],
                    in_=xt[0][:, cq * step0 : (cq + 1) * step0],
                )
            nc.sync.dma_start(out=cstf_t[:], in_=cstf[:, :])
            for i in range(1, N_TILES):
                q_eng[xq[i]].dma_start(out=xt_sb[i][:], in_=xt[i])
            # head constants are not needed until the first head chunk
            # (~wave 13): issue them after the whole x stream.
            nc.sync.dma_start(out=cstt[:, OFF_CE1:NCOL], in_=cst[:, OFF_CE1:NCOL])

            pooledP = persist.tile([98, G_HALF_PAD], bf16)  # A rows 0:34, B 64:98
            nc.vector.memset(pooledP[:, G_HALF:G_HALF_PAD], 0.0)
            ysb = persist.tile([2, HEAD_COLS], f32)

            layer_cfg = [
                (OFF_W1, 128, FB_B1, 128),
                (OFF_W2, 128, FB_B2, 128),
                (OFF_W3, 128, FB_B3, 128),
                (OFF_W4, 98, FB_B4, 98),
            ]

            def relu_op(eng, out_v, in_v, b_ap):
                if eng == "a":
                    nc.scalar.activation(out_v, in_v, Relu, bias=b_ap)
                else:
                    nc.vector.tensor_scalar(out_v, in_v, b_ap, 0.0, ADD, MAX)

            h_tiles = {}

            def mlp_layer(i, li, mps):
                """Emit matmuls + relus for (tile i, layer li)."""
                ncols = TILE_COLS if i < N_TILES - 1 else LAST_G * K
                cur = xt_sb[i] if li == 0 else h_tiles[(i, li - 1)]
                woff, wm, boff, outp = layer_cfg[li]
                w_ap = cstt[:, woff : woff + wm]
                b_ap = cstf_t[0:outp, boff : boff + 1]
                h = hp.tile([128, TILE_COLS], bf16, name="h")
                h_tiles[(i, li)] = h
                gstep = 1 if TUNE.get("singles", False) else 2
                for gi, c0 in enumerate(range(0, 4, gstep)):
                    lo = c0 * DATA_COLS
                    if lo >= ncols:
                        continue
                    ps = mps.tile(
                        [128, gstep * PS_STRIDE], mybir.dt.float32, name="ps",
                        tag="ps2",
                    )
                    dc_full = True
                    for g in range(gstep):
                        c = c0 + g
                        dc = min(DATA_COLS, ncols - c * DATA_COLS)
                        if dc <= 0:
                            dc_full = False
                            continue
                        if dc < DATA_COLS:
                            dc_full = False
                        nc.tensor.matmul(
                            ps[0:outp, g * PS_STRIDE : g * PS_STRIDE + dc],
                            w_ap,
                            cur[:, c * DATA_COLS : c * DATA_COLS + dc],
                            start=True,
                            stop=True,
                        )
                    eng = relu_eng_p[(i, li, gi)]
                    if dc_full:
                        ps_v = ps[0:outp, :].rearrange(
                            "p (c s) -> p c s", c=gstep
                        )[:, :, 0:DATA_COLS]
                        h_v = h[
                            0:outp, lo : lo + gstep * DATA_COLS
                        ].rearrange("p (c s) -> p c s", c=gstep)
                        relu_op(eng, h_v, ps_v, b_ap)
                    else:
                        for g in range(gstep):
                            c = c0 + g
                            dc = min(DATA_COLS, ncols - c * DATA_COLS)
                            if dc <= 0:
                                continue
                            relu_op(
                                eng,
                                h[0:outp, c * DATA_COLS : c * DATA_COLS + dc],
                                ps[0:outp, g * PS_STRIDE : g * PS_STRIDE + dc],
                                b_ap,
                            )

            def pool_tile(i):
                ng = TILE_G if i < N_TILES - 1 else LAST_G
                cur = h_tiles.pop((i, 3))
                h4v = cur[0:98, 0 : ng * K].rearrange("p (g k) -> p g k", k=K)
                hh = php.tile([98, TILE_G * (K // 2)], bf16, name="hh")
                hhv = hh[:, 0 : ng * (K // 2)].rearrange(
                    "p (g k) -> p g k", k=K // 2
                )
                pout = pooledP[:, i * TILE_G : i * TILE_G + ng]
                with nc.allow_low_precision(reason="pooled sums fit bf16"):
                    if TUNE["pool_modes"][i] == "p":
                        nc.gpsimd.tensor_tensor(
                            hhv, h4v[:, :, 0 : K // 2], h4v[:, :, K // 2 : K],
                            op=ADD,
                        )
                        hh2 = php.tile([98, TILE_G * 8], bf16, name="hh2")
                        hh2v = hh2[:, 0 : ng * 8].rearrange(
                            "p (g k) -> p g k", k=8
                        )
                        nc.gpsimd.tensor_tensor(
                            hh2v[:, :, 0:7], hhv[:, :, 0:7], hhv[:, :, 7:14],
                            op=ADD,
                        )
                        nc.gpsimd.tensor_copy(hh2v[:, :, 7:8], hhv[:, :, 14:15])
                        nc.vector.tensor_reduce(pout, hh2v, axis=AX, op=ADD)
                    else:
                        nc.vector.tensor_tensor(
                            hhv, h4v[:, :, 0 : K // 2], h4v[:, :, K // 2 : K],
                            op=ADD,
                        )
                        nc.vector.tensor_reduce(pout, hhv, axis=AX, op=ADD)

            conv1_cfg = [
                (OFF_CE1, OFF_CO1, 128),
                (OFF_CEM, OFF_COM, 128),
                (OFF_CE2, OFF_CO2, 112),
            ]
            conv2_cfg = [
                (OFF_C20, 0, 128, 128),
                (OFF_C21, 1, 128, 128),
                (OFF_C22, 2, 112, 96),
            ]

            def head_chunk(hps, half, c0, C):
                rhs = pooledP[half * 64 : half * 64 + 34, c0 : c0 + C]
                c1b = cstf_t[0:128, FB_C1B : FB_C1B + 1]
                c2b = cstf_t[0:128, FB_C2B : FB_C2B + 1]
                b0 = half * 64
                mts = []
                for bi, (offE, offO, m) in enumerate(conv1_cfg):
                    pp = hps.tile(
                        [128, 2 * PS_STRIDE], mybir.dt.float32, name="ps",
                        tag="ps2",
                    )
                    nc.tensor.matmul(
                        pp[0:m, 0:C], cstt[b0 : b0 + 34, offE : offE + m], rhs,
                        start=True, stop=True,
                    )
                    nc.tensor.matmul(
                        pp[0:m, PS_STRIDE : PS_STRIDE + C],
                        cstt[b0 : b0 + 34, offO : offO + m], rhs,
                        start=True, stop=True,
                    )
                    se = hsb.tile([128, 512], bf16, name=f"se{bi}")
                    nc.scalar.activation(
                        se[0:m, 0:C], pp[0:m, 0:C], Relu, bias=c1b[0:m, :]
                    )
                    ms = hsb.tile([128, 512], bf16, name=f"ms{bi}")
                    nc.vector.scalar_tensor_tensor(
                        ms[0:m, 0:C], pp[0:m, PS_STRIDE : PS_STRIDE + C],
                        c1b[0:m, :], se[0:m, 0:C], ADD, MAX,
                    )
                    mts.append(ms)
                rs = []
                for ci, (off, src, kk, m) in enumerate(conv2_cfg):
                    p2 = hps.tile(
                        [128, 2 * PS_STRIDE], mybir.dt.float32, name="ps",
                        tag="ps2",
                    )
                    nc.tensor.matmul(
                        p2[0:m, 0:C], cstt[0:kk, off : off + m],
                        mts[src][0:kk, 0:C],
                        start=True, stop=True,
                    )
                    r = hsb.tile([128, 512], bf16, name=f"r{ci}")
                    e = h2_eng_p[((half, c0), ci)]
                    if e == "a":
                        nc.scalar.activation(r[0:m, 0:C], p2[0:m, 0:C], Relu, bias=c2b[0:m, :])
                    elif e == "d":
                        nc.vector.tensor_scalar(r[0:m, 0:C], p2[0:m, 0:C], c2b[0:m, :], 0.0, ADD, MAX)
                    else:
                        nc.gpsimd.tensor_scalar(r[0:m, 0:C], p2[0:m, 0:C], c2b[0:m, :], 0.0, ADD, MAX)
                    rs.append((r, m))
                py = hps.tile(
                    [2, 2 * PS_STRIDE], mybir.dt.float32, name="ps", tag="ps2"
                )
                for gi, (off, (r, m)) in enumerate(
                    zip([OFF_WO0, OFF_WO1, OFF_WO2], rs)
                ):
                    nc.tensor.matmul(
                        py[:, 0:C], cstt[0:m, off : off + 2], r[0:m, 0:C],
                        start=(gi == 0), stop=(gi == 2),
                    )
                yc = ysb[:, half * G_HALF_PAD + c0 : half * G_HALF_PAD + c0 + C]
                e = cpy_eng_p[(half, c0)]
                if e == "a":
                    nc.scalar.add(yc, py[:, 0:C], add=0.0)
                elif e == "d":
                    nc.vector.tensor_scalar(yc, py[:, 0:C], 0.0, 0.0, ADD, ADD)
                else:
                    nc.gpsimd.tensor_scalar(yc, py[:, 0:C], 0.0, 0.0, ADD, ADD)

            head_at = TUNE["head_at"]
            with (
                tc.tile_pool(
                    name="mps", bufs=TUNE["ps_bufs"], space="PSUM"
                ) as mps,
                tc.tile_pool(name="php", bufs=2) as php,
            ):
                done = set()
                DEPTH = TUNE.get("wave_depth", 1)
                for w in range(N_TILES + 3 * DEPTH + 1):
                    for li in range(4):
                        i = w - li * DEPTH
                        if 0 <= i < N_TILES:
                            mlp_layer(i, li, mps)
                    ip = w - 3 * DEPTH - 1
                    if 0 <= ip < N_TILES:
                        pool_tile(ip)
                        for (hf, c0, C) in head_at.get(ip, []):
                            head_chunk(mps, hf, c0, C)
                            done.add((hf, c0))
                for hf, c0, C in TUNE.get("head_all", [
                    (0, 0, 256), (1, 0, 256), (0, 256, 256),
                    (1, 256, 256), (0, 512, 128), (1, 512, 128),
                ]):
                    if (hf, c0) not in done:
                        head_chunk(mps, hf, c0, C)
                nc.sync.dma_start(
                    out=y[:, 0:G_HALF_PAD], in_=ysb[:, 0:G_HALF_PAD]
                )
                nc.sync.dma_start(
                    out=y[:, G_HALF_PAD:HEAD_COLS],
                    in_=ysb[:, G_HALF_PAD:HEAD_COLS],
                )
    return nc


def _get_nc():
    if "nc" not in _NC_CACHE:
        _NC_CACHE["nc"] = _build_nc()
    return _NC_CACHE["nc"]


# ------------------------------------------------------------------ host prep
def _prep_x(x):
    """[N, 64] fp32 -> per-core [N_TILES, 128, 1920] bf16 transposed tiles."""
    import ml_dtypes

    xs = np.ascontiguousarray(x.reshape(G, NODES_PER_G, F)[:, :K, :])
    xs = xs.astype(ml_dtypes.bfloat16)
    out = np.zeros((NCORE, N_TILES, 128, TILE_COLS), ml_dtypes.bfloat16)
    for c in range(NCORE):
        for half in range(2):
            gs = c * G_CORE + half * G_HALF
            segp = np.zeros((G_HALF_PAD, K, F), ml_dtypes.bfloat16)
            segp[:G_HALF] = xs[gs : gs + G_HALF]
            # [tiles, 64 graphs, 30, F] -> [tiles, F, 64*30]
            a = segp.reshape(N_TILES, TILE_G * K, F)
            out[c][:, half * F : (half + 1) * F, :] = a.transpose(0, 2, 1)
    return out


def _build_const(W1, b1, W2, b2, W3, b3, W4, b4, cw1, cb1, cw2, cb2, Wo, bo):
    import ml_dtypes

    cst = np.zeros((128, NCOL), np.float32)

    def bd(W):  # torch [out, in] -> block-diag lhsT [128, 2*out]
        o = W.shape[0]
        m = np.zeros((128, 2 * o), np.float32)
        m[0:64, 0:o] = W.T
        m[64:128, o : 2 * o] = W.T
        return m

    cst[:, OFF_W1 : OFF_W1 + 128] = bd(W1)
    cst[:, OFF_W2 : OFF_W2 + 128] = bd(W2)
    cst[:, OFF_W3 : OFF_W3 + 128] = bd(W3)
    w4m = np.zeros((128, 98), np.float32)
    w4m[0:64, 0:34] = W4.T
    w4m[64:128, 64:98] = W4.T
    cst[:, OFF_W4 : OFF_W4 + 98] = w4m

    def conv1_lhsT(ts):  # [34, 16*len(ts)]; includes the 1/30 mean fold
        m = np.zeros((34, 16 * len(ts)), np.float32)
        for ul, t in enumerate(ts):
            for oc in range(16):
                m[t : t + 5, ul * 16 + oc] = cw1[oc, 0, :] / float(K)
        return m

    # conv1 lhsT blocks live at partitions 0:34 (half A) AND 64:98 (half B)
    # so the head matmul lhsT base partition matches its pooled rhs.
    for r0 in (0, 64):
        cst[r0 : r0 + 34, OFF_CE1 : OFF_CE1 + 128] = conv1_lhsT(range(0, 16, 2))
        cst[r0 : r0 + 34, OFF_CO1 : OFF_CO1 + 128] = conv1_lhsT(range(1, 16, 2))
        cst[r0 : r0 + 34, OFF_CEM : OFF_CEM + 128] = conv1_lhsT(range(8, 24, 2))
        cst[r0 : r0 + 34, OFF_COM : OFF_COM + 128] = conv1_lhsT(range(9, 24, 2))
        cst[r0 : r0 + 34, OFF_CE2 : OFF_CE2 + 112] = conv1_lhsT(range(16, 30, 2))
        cst[r0 : r0 + 34, OFF_CO2 : OFF_CO2 + 112] = conv1_lhsT(range(17, 30, 2))

    def conv2_lhsT(tgs, us):  # [16*len(us), 32*len(tgs)]
        m = np.zeros((16 * len(us), 32 * len(tgs)), np.float32)
        for ri, u in enumerate(us):
            for ci, t in enumerate(tgs):
                kk = u - t
                if 0 <= kk < 5:
                    for ic in range(16):
                        m[ri * 16 + ic, ci * 32 : (ci + 1) * 32] = cw2[:, ic, kk]
        return m

    cst[0:128, OFF_C20 : OFF_C20 + 128] = conv2_lhsT(range(0, 4), range(0, 8))
    cst[0:128, OFF_C21 : OFF_C21 + 128] = conv2_lhsT(range(4, 8), range(4, 12))
    cst[0:112, OFF_C22 : OFF_C22 + 96] = conv2_lhsT(range(8, 11), range(8, 15))

    def wo_map(ts):  # [32*len(ts), 2]; undo the oc2-major flatten order
        m = np.zeros((32 * len(ts), 2), np.float32)
        for ci, t in enumerate(ts):
            for oc2 in range(32):
                m[ci * 32 + oc2, :] = Wo[:, oc2 * 11 + t]
        return m

    cst[0:128, OFF_WO0 : OFF_WO0 + 2] = wo_map(range(0, 4))
    cst[0:128, OFF_WO1 : OFF_WO1 + 2] = wo_map(range(4, 8))
    cst[0:96, OFF_WO2 : OFF_WO2 + 2] = wo_map(range(8, 11))

    cstf = np.zeros((128, 8), np.float32)
    cstf[0:128, FB_B1] = np.concatenate([b1, b1])
    cstf[0:128, FB_B2] = np.concatenate([b2, b2])
    cstf[0:128, FB_B3] = np.concatenate([b3, b3])
    cstf[0:34, FB_B4] = b4
    cstf[64:98, FB_B4] = b4
    cstf[0:128, FB_C1B] = np.tile(cb1, 8)
    cstf[0:128, FB_C2B] = np.tile(cb2, 4)
    return cst.astype(ml_dtypes.bfloat16), cstf


def _numpy_forward(x, batch, W1, b1, W2, b2, W3, b3, W4, b4, cw1, cb1, cw2, cb2, Wo, bo):
    """General (slow) host fallback, used only if batch is not arange//100."""
    h = np.maximum(x @ W1.T + b1, 0)
    h = np.maximum(h @ W2.T + b2, 0)
    h = np.maximum(h @ W3.T + b3, 0)
    h = np.maximum(h @ W4.T + b4, 0)
    counts = np.bincount(batch, minlength=G).astype(np.float32)
    starts = np.cumsum(counts) - counts
    pos = np.arange(h.shape[0], dtype=np.float32) - starts[batch]
    mask = (pos < K).astype(np.float32)
    sums = np.zeros((G, h.shape[1]), np.float32)
    np.add.at(sums, batch, h * mask[:, None])
    denom = np.minimum(counts, float(K))
    pooled = sums / denom[:, None]
    c1 = np.zeros((G, 16, 30), np.float32)
    for t in range(30):
        c1[:, :, t] = pooled[:, t : t + 5] @ cw1[:, 0, :].T
    c1 = np.maximum(c1 + cb1[None, :, None], 0)
    m = np.maximum(c1[:, :, 0::2], c1[:, :, 1::2])  # [G, 16, 15]
    c2 = np.zeros((G, 32, 11), np.float32)
    for t in range(11):
        c2[:, :, t] = np.einsum("gik,oik->go", m[:, :, t : t + 5], cw2)
    c2 = np.maximum(c2 + cb2[None, :, None], 0)
    flat = c2.reshape(G, -1)
    return flat @ Wo.T + bo


def _run(inputs, trace=False, trace_kwargs=None):
    """Returns (y [10000, 2], BassKernelResults-or-None)."""
    x = np.ascontiguousarray(np.asarray(inputs["x"], dtype=np.float32))
    batch = np.asarray(inputs["batch"])
    names = ["W1", "b1", "W2", "b2", "W3", "b3", "W4", "b4",
             "cw1", "cb1", "cw2", "cb2", "Wo", "bo"]
    ws = [np.ascontiguousarray(np.asarray(inputs[n], dtype=np.float32)) for n in names]

    expected_batch = (np.arange(N, dtype=np.int64) // (N // G)).astype(batch.dtype)
    if batch.shape != (N,) or not np.array_equal(batch, expected_batch):
        return _numpy_forward(x, np.asarray(batch, np.int64), *ws), None

    from concourse.bass_utils import run_bass_kernel_spmd

    nc = _get_nc()
    xt_all = _prep_x(x)
    cst, cstf = _build_const(*ws)
    in_maps = [{"xt": xt_all[c], "cst": cst, "cstf": cstf} for c in range(NCORE)]
    kw = {}
    if trace:
        kw["trace"] = True
        if trace_kwargs:
            kw["trace_kwargs"] = trace_kwargs
    res = run_bass_kernel_spmd(nc, in_maps, core_ids=list(range(NCORE)), **kw)

    bo = ws[-1]
    out = np.empty((G, 2), np.float32)
    for c in range(NCORE):
        yc = np.asarray(res.results[c]["y"], np.float32)
        base = c * G_CORE
        out[base : base + G_HALF] = yc[:, 0:G_HALF].T
        out[base + G_HALF : base + G_CORE] = yc[
            :, G_HALF_PAD : G_HALF_PAD + G_HALF
        ].T
    return out + bo[None, :], res


def kernel(**inputs) -> np.ndarray:
    out, _ = _run(inputs)
    return out

